# revision 25
# baseline (speedup 1.0000x reference)
"""Multi-head attention (B=4, S=2048, D=1024, H=16) on 8 trn2 NeuronCores.

Sharding (tensor-parallel on heads, data-parallel on batch): core c handles
batch b = c//2 and head-group g = c%2 (8 of the 16 heads).  Each core emits a
PARTIAL output [2048, 1024]; the host sums the two partials per batch.

Kernel layout ("transposed scores"): QT/KT = [d_head on partitions, seq] so
scoresT = [k_seq partitions, q_seq free]; softmax row-sum comes free from a
ones-column appended to V (M=65 context matmul); exp() on ScalarE reads PSUM
directly.  All matmuls bf16 with fp32 PSUM.

v2 changes vs baseline:
- Score matmuls for the two heads of a pair are emitted adjacently; their
  K=64 contractions sit in disjoint PE row-groups (partitions 0-63 / 64-127)
  so the hardware runs them concurrently (~2x on the score phase).
- Activation chunks (q/k/v) are loaded from HBM once per 512-seq chunk and
  projected for all head pairs (baseline re-loaded q,k 4x).
- Rolling cross-unit software pipeline: context matmuls trail their scores
  by LAG global steps, so there is never a ctx-only drain burst; projection
  and out-projection runs are injected into scheduled slots.
- cx PSUM banks are freed immediately after the last context matmul by two
  fast DVE copies to SBUF staging; the slow (3.3us) exact reciprocal and
  the normalize multiply run later in DVE idle time via a deferred queue
  (deadline: the out-projections of the next chunk).
- Out-projection bias via a host-replicated [128,1024] tile + DVE add,
  removing the 32 ONES bias matmuls.
"""

import numpy as np
import ml_dtypes

B, S, D = 4, 2048, 1024
H, DH = 16, 64
NCORES = 8
P = 128
KT_IN = D // P       # 8 contraction tiles for the projections
HL = H // 2          # 8 local heads per core
DL = HL * DH         # 512 local projection columns
NPAIR = HL // 2      # 4 local head pairs
MT = DL // P         # 4 m-tiles for QT/KT projections
NKT = S // P         # 16 k-tiles in the attention contraction
QCH = S // 512       # 4 q-chunks
VW = DH + 1          # 65: V columns per head incl. the ones column

BF16 = ml_dtypes.bfloat16

_NC_CACHE = {}


def _build_nc():
    import concourse.bass as bass
    import concourse.mybir as mybir
    import concourse.tile as tile
    from concourse import bacc
    from contextlib import ExitStack

    dt = mybir.dt
    F32, BF = dt.float32, dt.bfloat16
    AF = mybir.ActivationFunctionType
    ALU = mybir.AluOpType

    nc = bacc.Bacc(None)

    qT_d = nc.dram_tensor("qT", [KT_IN, P, S], BF, kind="ExternalInput")
    kT_d = nc.dram_tensor("kT", [KT_IN, P, S], BF, kind="ExternalInput")
    vT_d = nc.dram_tensor("vT", [KT_IN, P, S], BF, kind="ExternalInput")
    wq_d = nc.dram_tensor("wq", [KT_IN, P, DL], BF, kind="ExternalInput")
    wk_d = nc.dram_tensor("wk", [KT_IN, P, DL], BF, kind="ExternalInput")
    wv_d = nc.dram_tensor("wv", [KT_IN, P, DL], BF, kind="ExternalInput")
    wo_d = nc.dram_tensor("wo", [P, NPAIR, D], BF, kind="ExternalInput")
    bq_d = nc.dram_tensor("bq", [P, MT], F32, kind="ExternalInput")
    bk_d = nc.dram_tensor("bk", [P, MT], F32, kind="ExternalInput")
    bo_d = nc.dram_tensor("bo", [P, D], F32, kind="ExternalInput")
    out_d = nc.dram_tensor("out", [S, D], F32, kind="ExternalOutput")

    with tile.TileContext(nc) as tc, ExitStack() as ctx:
        persist = ctx.enter_context(tc.tile_pool(name="persist", bufs=1))
        wpool = ctx.enter_context(tc.tile_pool(name="wpool", bufs=1))
        qstream = ctx.enter_context(tc.tile_pool(name="qstream", bufs=2))
        kstream = ctx.enter_context(tc.tile_pool(name="kstream", bufs=4))
        vstream = ctx.enter_context(tc.tile_pool(name="vstream", bufs=2))
        ptpool = ctx.enter_context(tc.tile_pool(name="ptpool", bufs=10))
        stpool = ctx.enter_context(tc.tile_pool(name="stpool", bufs=2))
        small = ctx.enter_context(tc.tile_pool(name="small", bufs=2))
        ostream = ctx.enter_context(tc.tile_pool(name="ostream", bufs=3))
        psum_sc = ctx.enter_context(
            tc.tile_pool(name="psum_sc", bufs=2, space="PSUM")
        )
        psum_cx = ctx.enter_context(
            tc.tile_pool(name="psum_cx", bufs=2, space="PSUM")
        )
        psum_pr = ctx.enter_context(
            tc.tile_pool(name="psum_pr", bufs=2, space="PSUM")
        )

        # ---- persistent SBUF tensors
        QT = persist.tile([P, MT, S], BF)          # [128(d of pair), pair, q]
        KT = persist.tile([P, MT, S], BF)          # [128(d of pair), pair, k]
        VA = persist.tile([P, NKT, HL * VW], BF)   # [128(k), s-tile, h*65 + d]
        CT = persist.tile([P, NPAIR, S], BF)       # ctxT, pair-packed rows

        BQ = wpool.tile([P, MT], F32)
        BK = wpool.tile([P, MT], F32)
        WO = wpool.tile([P, NPAIR, D], BF)
        BOR = wpool.tile([P, D], F32)              # replicated out bias
        WQ = wpool.tile([P, KT_IN, DL], BF)
        WK = wpool.tile([P, KT_IN, DL], BF)
        WV = wpool.tile([P, KT_IN, DL], BF)

        # ones columns of V_aug (col 64 of each head's 65-wide block)
        va_h = VA[:].rearrange("p t (h e) -> p t h e", e=VW)
        nc.vector.memset(va_h[:, :, :, DH : DH + 1], 1.0)

        def load_weights_early():
            nc.sync.dma_start(WK, wk_d[:].rearrange("k p d -> p k d"))
            nc.sync.dma_start(WQ, wq_d[:].rearrange("k p d -> p k d"))
            nc.sync.dma_start(BK, bk_d[:])
            nc.sync.dma_start(BQ, bq_d[:])

        def load_weights_late():
            nc.sync.dma_start(WV, wv_d[:].rearrange("k p d -> p k d"))
            nc.sync.dma_start(WO, wo_d[:])
            nc.sync.dma_start(BOR, bo_d[:])

        # ---- chunk activation tiles (loaded once per chunk)
        qs_t = {}
        ks_t = {}
        vs_t = {}

        def load_chunk(kind, c):
            pool, dram, store = {
                "q": (qstream, qT_d, qs_t),
                "k": (kstream, kT_d, ks_t),
                "v": (vstream, vT_d, vs_t),
            }[kind]
            t = pool.tile([P, KT_IN, 512], BF, tag=kind + "s")
            nc.sync.dma_start(
                t,
                dram[:, :, c * 512 : (c + 1) * 512].rearrange("k p s -> p k s"),
            )
            store[c] = t

        def qk_proj(which, j, c):
            """QT/KT m-tile j from already-loaded chunk c."""
            src = qs_t[c] if which == "q" else ks_t[c]
            W = WQ if which == "q" else WK
            Bb = BQ if which == "q" else BK
            dst = QT if which == "q" else KT
            ps = psum_pr.tile([P, 512], F32, tag="prps")
            for kt in range(KT_IN):
                nc.tensor.matmul(
                    ps,
                    lhsT=W[:, kt, j * P : (j + 1) * P],
                    rhs=src[:, kt, :],
                    start=(kt == 0),
                    stop=(kt == KT_IN - 1),
                )
            nc.vector.tensor_tensor(
                dst[:, j, c * 512 : (c + 1) * 512],
                ps,
                Bb[:, j : j + 1].to_broadcast([P, 512]),
                ALU.add,
            )

        def v_proj(st):
            """V rows for global seq-tile st (128 positions) -> VA[:, st]."""
            c, sub = divmod(st, 4)
            vs = vs_t[c]
            ps = psum_pr.tile([P, 512], F32, tag="prps")
            for kt in range(KT_IN):
                nc.tensor.matmul(
                    ps,
                    lhsT=vs[:, kt, sub * P : (sub + 1) * P],
                    rhs=WV[:, kt, :],
                    start=(kt == 0),
                    stop=(kt == KT_IN - 1),
                )
            dst = va_h[:, st, :, 0:DH]
            nc.vector.tensor_copy(dst, ps[:].rearrange("p (h e) -> p h e", e=DH))

        def out_block(qt, ec):
            """Out-projection rows qt*128.. for output cols ec*512.."""
            ps = psum_pr.tile([P, 512], F32, tag="prps")
            for jj in range(NPAIR):
                nc.tensor.matmul(
                    ps,
                    lhsT=CT[:, jj, qt * P : (qt + 1) * P],
                    rhs=WO[:, jj, ec * 512 : (ec + 1) * 512],
                    start=(jj == 0),
                    stop=(jj == NPAIR - 1),
                )
            ot = ostream.tile([P, 512], F32, tag="ot")
            nc.vector.tensor_tensor(
                ot, ps, BOR[:, ec * 512 : (ec + 1) * 512], ALU.add
            )
            nc.sync.dma_start(
                out_d[qt * P : (qt + 1) * P, ec * 512 : (ec + 1) * 512], ot
            )

        inv_sqrt_dh = 1.0 / float(np.sqrt(DH))

        LAG = 8  # ctx matmuls trail their scores by LAG global steps.

        # Rolling cross-unit pipeline state.
        from collections import deque

        ctxq = deque()   # pending ctx steps: (state, kt, pt)
        finq = []        # deferred normalize thunks: [delay_steps, thunk]

        def tick_finq():
            for e in finq:
                e[0] -= 1
            while finq and finq[0][0] <= 0:
                finq.pop(0)[1]()

        USE_SHARED_LDW = False

        def scores_step(st, kt):
            sc = psum_sc.tile([P, 2, 512], F32, tag="scps")
            j, c = st["j"], st["c"]
            if USE_SHARED_LDW:
                # one full 128x128 LDWEIGHTS carries BOTH heads' stationary
                # K-tiles (rows 0-63 = head A, 64-127 = head B); the two
                # K=64 matmuls skip their own weight loads and execute
                # concurrently in disjoint row-groups.
                nc.tensor.ldweights(KT[:, j, kt * P : (kt + 1) * P])
                for v in (0, 1):
                    rows = slice(0, DH) if v == 0 else slice(DH, P)
                    rhs = QT[rows, j, c * 512 : (c + 1) * 512]
                    ifmap_ap = nc.tensor.lower_ap(rhs.opt({0}), opt=False)
                    out_ap = nc.tensor.lower_ap(sc[:, v, :])
                    nc.tensor.add_instruction(
                        mybir.InstMatmult(
                            name=nc.get_next_instruction_name(),
                            replication_resolution=0,
                            replication_shift_amnt=0,
                            replication_num_rows=0,
                            start_tensor_calc=True,
                            stop_tensor_calc=True,
                            ins=[ifmap_ap],
                            outs=[out_ap],
                            perf_mode=None,
                            is_transpose=None,
                            ifmap_quant_offset=None,
                            weights_quant_offset=None,
                            bass_skip_group_check=False,
                            tile_position=(0 if v == 0 else DH, 0),
                            tile_size=(DH, P),
                        )
                    )
            else:
                nc.tensor.matmul(
                    sc[:, 0, :],
                    lhsT=KT[0:DH, j, kt * P : (kt + 1) * P],
                    rhs=QT[0:DH, j, c * 512 : (c + 1) * 512],
                    start=True,
                    stop=True,
                )
                nc.tensor.matmul(
                    sc[:, 1, :],
                    lhsT=KT[DH:P, j, kt * P : (kt + 1) * P],
                    rhs=QT[DH:P, j, c * 512 : (c + 1) * 512],
                    start=True,
                    stop=True,
                )
            pt = ptpool.tile([P, 2, 512], BF, tag="pt")
            nc.scalar.activation(pt, sc, AF.Exp, scale=inv_sqrt_dh)
            ctxq.append((st, kt, pt))

        def normalize_head(st, v):
            """Normalize one head's staged context into CT (deferred)."""
            stg65 = st["stgA"] if v == 0 else st["stgB"]
            j, c = st["j"], st["c"]
            rec = small.tile([1, 512], F32, tag="rec")
            nc.vector.reciprocal(rec, stg65[DH : DH + 1, :])
            recb = small.tile([DH, 512], F32, tag="recb")
            nc.gpsimd.partition_broadcast(recb, rec)
            if v == 0:
                nc.vector.tensor_tensor(
                    CT[0:DH, j, c * 512 : (c + 1) * 512],
                    stg65[0:DH, :],
                    recb,
                    ALU.mult,
                )
            else:
                stg = small.tile([DH, 512], BF, tag="stg")
                nc.vector.tensor_tensor(stg, stg65[0:DH, :], recb, ALU.mult)
                nc.sync.dma_start(CT[DH:P, j, c * 512 : (c + 1) * 512], stg)

        def ctx_step():
            st, kt, pt = ctxq.popleft()
            hA, hB = st["hA"], st["hB"]
            nc.tensor.matmul(
                st["cxA"][0 : DH + 1, :],
                lhsT=VA[:, kt, VW * hA : VW * hA + VW],
                rhs=pt[:, 0, :],
                start=(kt == 0),
                stop=(kt == NKT - 1),
            )
            nc.tensor.matmul(
                st["cxB"][0 : DH + 1, :],
                lhsT=VA[:, kt, VW * hB : VW * hB + VW],
                rhs=pt[:, 1, :],
                start=(kt == 0),
                stop=(kt == NKT - 1),
            )
            if kt == NKT - 1:
                # free the cx banks fast with plain copies; the slow
                # reciprocal+normalize runs later in DVE idle time.
                stgA = stpool.tile([DH + 1, 512], F32, tag="st65")
                nc.vector.tensor_copy(stgA, st["cxA"][0 : DH + 1, :])
                stgB = stpool.tile([DH + 1, 512], F32, tag="st65")
                nc.vector.tensor_copy(stgB, st["cxB"][0 : DH + 1, :])
                st["stgA"], st["stgB"] = stgA, stgB
                finq.append([2, lambda: normalize_head(st, 0)])
                finq.append([4, lambda: normalize_head(st, 1)])

        # ---- injection schedule -------------------------------------------
        # Per-unit work. Encoding:
        #   ("kp", j, c) -> qk_proj("k", j, c)   ("qp", j, c) -> qk_proj("q",.)
        #   ("v", st)    -> v_proj(st)           ("lq"/"lk"/"lv", c) -> DMA
        #   ("ob", qt, ec) -> out_block
        # Value is either a list (spread uniformly over the 16 slots) or a
        # dict {slot: [items]} for explicit placement.
        SCHED = {
            # unit (0,0): stream in the rest of K/V chunks + all V tiles,
            # paced just ahead of their consumers.
            (0, 0): {0: [("lk", 1), ("lv", 1), ("v", 0)], 1: [("v", 1)],
                     2: [("kp", 0, 1), ("v", 2)], 3: [("v", 3)],
                     4: [("lk", 2), ("v", 4)], 5: [("kp", 0, 2), ("v", 5)],
                     6: [("lk", 3), ("v", 6)], 7: [("lv", 2), ("v", 7)],
                     8: [("kp", 0, 3)], 9: [("v", 8)],
                     10: [("v", 9)], 11: [("v", 10)], 12: [("v", 11)],
                     13: [("lv", 3), ("qp", 1, 0)], 14: [("kp", 1, 0)]},
            (1, 0): {0: [("kp", 1, 1)], 1: [("v", 12)], 2: [("v", 13)],
                     3: [("v", 14)], 4: [("v", 15)], 6: [("kp", 1, 2)],
                     8: [("kp", 1, 3)], 10: [("qp", 2, 0)],
                     12: [("kp", 2, 0)]},
            (2, 0): [("kp", 2, 1), ("kp", 2, 2), ("kp", 2, 3),
                     ("qp", 3, 0), ("kp", 3, 0)],
            (3, 0): [("kp", 3, 1), ("kp", 3, 2), ("kp", 3, 3),
                     ("lq", 1), ("qp", 0, 1), ("qp", 1, 1)],
            (0, 1): {4: [("qp", 2, 1)], 8: [("qp", 3, 1)],
                     12: [("ob", 0, 0)]},
            (1, 1): [("lq", 2), ("ob", 0, 1), ("ob", 1, 0)],
            (2, 1): [("qp", 0, 2), ("qp", 1, 2), ("ob", 1, 1), ("ob", 2, 0)],
            (3, 1): [("qp", 2, 2), ("qp", 3, 2), ("ob", 2, 1), ("ob", 3, 0)],
            (0, 2): {4: [("lq", 3)], 8: [("ob", 3, 1)]},
            (1, 2): [("qp", 0, 3), ("qp", 1, 3), ("ob", 4, 0), ("ob", 4, 1)],
            (2, 2): [("qp", 2, 3), ("qp", 3, 3), ("ob", 5, 0), ("ob", 5, 1)],
            (3, 2): [("ob", 6, 0), ("ob", 6, 1), ("ob", 7, 0)],
            (0, 3): {8: [("ob", 7, 1)]},
            (1, 3): [("ob", 8, 0), ("ob", 8, 1), ("ob", 9, 0)],
            (2, 3): [("ob", 9, 1), ("ob", 10, 0), ("ob", 10, 1)],
            (3, 3): [("ob", 11, 0), ("ob", 11, 1)],
        }

        def run_item(it):
            kind = it[0]
            if kind == "kp":
                qk_proj("k", it[1], it[2])
            elif kind == "qp":
                qk_proj("q", it[1], it[2])
            elif kind == "v":
                v_proj(it[1])
            elif kind == "lq":
                load_chunk("q", it[1])
            elif kind == "lk":
                load_chunk("k", it[1])
            elif kind == "lv":
                load_chunk("v", it[1])
            elif kind == "ob":
                out_block(it[1], it[2])

        def unit_slots(j, c):
            sched = SCHED.get((j, c), [])
            slots = [[] for _ in range(NKT)]
            if isinstance(sched, dict):
                for s, items in sched.items():
                    slots[s].extend(items)
            else:
                n = len(sched)
                for i, it in enumerate(sched):
                    slots[(i * NKT) // n if n else 0].append(it)
            return slots

        # ---- emission ------------------------------------------------------
        # prologue: exactly what unit (0,0)'s start needs, loaded first.
        load_chunk("k", 0)
        load_chunk("q", 0)
        load_weights_early()
        load_chunk("v", 0)
        load_weights_late()
        qk_proj("k", 0, 0)
        qk_proj("q", 0, 0)

        for u in range(NPAIR * QCH):
            j, c = u % NPAIR, u // NPAIR
            cxA = psum_cx.tile([P, 512], F32, tag="cxps")
            cxB = psum_cx.tile([P, 512], F32, tag="cxps")
            st = {
                "j": j, "c": c, "hA": 2 * j, "hB": 2 * j + 1,
                "cxA": cxA, "cxB": cxB,
            }
            slots = unit_slots(j, c)
            for kt in range(NKT):
                tick_finq()
                for it in slots[kt]:
                    run_item(it)
                scores_step(st, kt)
                if len(ctxq) > LAG:
                    ctx_step()

        # drain the pipeline
        while ctxq:
            ctx_step()
        while finq:
            finq.pop(0)[1]()

        # tail: out-projection for the last chunk
        for qt in range(4 * (QCH - 1), 4 * QCH):
            for ec in range(2):
                out_block(qt, ec)

    nc.compile()
    return nc


def _get_nc():
    if "nc" not in _NC_CACHE:
        _NC_CACHE["nc"] = _build_nc()
    return _NC_CACHE["nc"]


def kernel(query, key, value, Wq, bq, Wk, bk, Wv, bv, Wo, bo):
    from concourse.bass_utils import run_bass_kernel_spmd

    query = np.asarray(query, dtype=np.float32)
    key = np.asarray(key, dtype=np.float32)
    value = np.asarray(value, dtype=np.float32)
    Wq = np.asarray(Wq, dtype=np.float32)
    Wk = np.asarray(Wk, dtype=np.float32)
    Wv = np.asarray(Wv, dtype=np.float32)
    Wo = np.asarray(Wo, dtype=np.float32)
    bq = np.asarray(bq, dtype=np.float32)
    bk = np.asarray(bk, dtype=np.float32)
    bv = np.asarray(bv, dtype=np.float32)
    bo = np.asarray(bo, dtype=np.float32)

    nc = _get_nc()

    # per-head-group weight shards
    shards = []
    for g in range(2):
        cols = slice(DL * g, DL * (g + 1))
        wq_t = np.ascontiguousarray(
            Wq.reshape(KT_IN, P, D)[:, :, cols]).astype(BF16)
        wk_t = np.ascontiguousarray(
            Wk.reshape(KT_IN, P, D)[:, :, cols]).astype(BF16)
        wv_t = np.ascontiguousarray(
            Wv.reshape(KT_IN, P, D)[:, :, cols]).astype(BF16)
        # Wo rows (hl*64+d) of this group -> [ (v,d)=128, local pair j, e ]
        wo_p = np.ascontiguousarray(
            Wo[cols, :].reshape(NPAIR, 2, DH, D).transpose(1, 2, 0, 3)
            .reshape(P, NPAIR, D)).astype(BF16)
        bq_t = np.ascontiguousarray(bq[cols].reshape(MT, P).T).astype(np.float32)
        bk_t = np.ascontiguousarray(bk[cols].reshape(MT, P).T).astype(np.float32)
        # attn rows sum to 1 => this group's V bias contributes bv_g @ Wo_g;
        # the global bo is added by the g=0 core only (host sums partials).
        bo_eff = bv[cols].astype(np.float64) @ Wo[cols, :].astype(np.float64)
        if g == 0:
            bo_eff = bo_eff + bo.astype(np.float64)
        bo_rep = np.tile(bo_eff[None, :], (P, 1)).astype(np.float32)
        shards.append({
            "wq": wq_t, "wk": wk_t, "wv": wv_t, "wo": wo_p,
            "bq": bq_t, "bk": bk_t, "bo": bo_rep,
        })

    in_maps = []
    per_batch = {}
    for c in range(NCORES):
        b, g = divmod(c, 2)
        if b not in per_batch:
            per_batch[b] = {
                "qT": np.ascontiguousarray(query[b].T).reshape(
                    KT_IN, P, S).astype(BF16),
                "kT": np.ascontiguousarray(key[b].T).reshape(
                    KT_IN, P, S).astype(BF16),
                "vT": np.ascontiguousarray(value[b].T).reshape(
                    KT_IN, P, S).astype(BF16),
            }
        in_maps.append({**shards[g], **per_batch[b]})

    _NC_CACHE["last_in_maps"] = in_maps
    globals()["_LAST_IN_MAPS"] = in_maps
    res = run_bass_kernel_spmd(nc, in_maps, core_ids=list(range(NCORES)))

    out = np.empty((B, S, D), np.float32)
    for b in range(B):
        out[b] = res.results[2 * b]["out"] + res.results[2 * b + 1]["out"]
    return out
